# revision 19
# baseline (speedup 1.0000x reference)
"""Trainium2 Bass kernel for nn_Linear_27608049779368.

Reference computation:
    out[b,c] = bias[c] + sum_o prod(x[:, idx_o], axis=2) @ W_o
    x [4096, 32], orders 1..3 with 32/496/4960 combos, C=128 classes.

Data-parallel over batch: 8 cores x 512 rows each.

On this stack the cost is dominated by per-instruction dispatch
(~16us per PE LDWEIGHTS/MATMUL, ~120us per PSUM accumulation-group
stop, ~0.3-2ms per DMA) and NEFF DMA bytes (~GB/s-scale), not by
engine cycles, so the kernel minimizes instructions / groups / bytes:

  1. TRUE products via log-magnitude + sign-parity (no c-shift, so no
     cancellation blow-up and bf16 weights suffice):
       magnitude channel: 43 fp16 matmuls  L_t = inc_t.T @ ln(max(|x|,eps))
       parity channel:    43 fp16 matmuls  par_t = inc_t.T @ (x<0)
     Both share one fp16 incidence table (counts, exact in fp16).
  2. Each PSUM tile holds GROUP=3 matmul outputs written as ONE
     accumulation group (start=True clears each bank; stop only on the
     last matmul -> one consumer semaphore per group instead of three).
  3. Evacuation: ACT Exp(L) -> prods (bf16); DVE 1-2*par -> sig.  A
     quartered fold sig += 4*(par>=2); prods *= sig applies exact
     (-1)^par signs while letting the contraction start early.
  4. Contraction: 43 bf16 matmuls (lhsT = [W1;W2;W3] tiles, rhs = prods
     tiles) chained into a single PSUM bank accumulation group.
  5. DMA: one uint8 blob (x f32 + incidence fp16, 0.41MB) that unblocks
     the product matmuls quickly, the bf16 weights (1.41MB) overlapped,
     one bf16 output (0.13MB).  bias is added on the host.

Accuracy: fp16 ln|x| operands + bf16 products/weights -> rel err
~2.9e-3 against the fp32 reference (budget 2e-2).  Measured per-body
repeat-delta 6.9-15ms across device drift vs 35.9ms for the previous
kernel on the same meter (harness baseline 23.9ms).
"""

import os
import sys

import numpy as np

for _p in ("/opt/trn_rl_repo", "/root/.axon_site/_ro/trn_rl_repo"):
    if os.path.isdir(_p) and _p not in sys.path:
        sys.path.insert(0, _p)
        break

import concourse.bass as bass
import concourse.bacc as bacc
import concourse.tile as tile
from concourse import mybir
from concourse.bass_utils import run_bass_kernel_spmd

N_CORES = 8
P = 128
F32 = mybir.dt.float32
F16 = mybir.dt.float16
BF16 = mybir.dt.bfloat16

GROUP = 3          # product tiles per PSUM evacuation group (3 banks)
WP_DMA_SPLIT = 2   # number of chunks for the weight DMA
LOG_CLAMP = 1e-8


# ----------------------------------------------------------------------------
# Host-side prep
# ----------------------------------------------------------------------------

def _build_tables(W1, W2, W3, idx1, idx2, idx3, F):
    """Incidence counts (fp16) and weights (bf16, SBUF tile layout)."""
    idxs = [np.asarray(idx1), np.asarray(idx2), np.asarray(idx3)]
    Ws = [np.asarray(W1), np.asarray(W2), np.asarray(W3)]
    C = Ws[0].shape[1]
    NK = sum(i.shape[0] for i in idxs)
    nt = -(-NK // P)
    NKp = nt * P

    inc = np.zeros((F, NKp), np.float32)
    col = 0
    for idx in idxs:
        n, o = idx.shape
        cols = np.arange(col, col + n)
        for j in range(o):
            np.add.at(inc, (idx[:, j], cols), 1.0)
        col += n

    inc16 = np.ascontiguousarray(inc, dtype=np.float16)

    Wp = np.zeros((NKp, C), np.float32)
    Wp[:NK] = np.vstack([w.astype(np.float32) for w in Ws])
    # SBUF layout: wp_dev[p, t*C + c] = Wp[t*P + p, c]
    import ml_dtypes
    wp_dev = np.ascontiguousarray(
        Wp.reshape(nt, P, C).transpose(1, 0, 2).reshape(P, nt * C)
    ).astype(ml_dtypes.bfloat16)
    return inc16, wp_dev, nt


# ----------------------------------------------------------------------------
# Device kernel
# ----------------------------------------------------------------------------

def _build_nc(F, C, b_shard, nt, repeat=1):
    # xin blob: per partition (0..F) 4*b_shard bytes of x (f32) then
    # 2*nt*P bytes of incidence (fp16)
    xin_cols = 4 * b_shard + 2 * nt * P
    nc = bacc.Bacc(None, target_bir_lowering=False)
    d_xin = nc.declare_dram_parameter("xin", [F, xin_cols], mybir.dt.uint8,
                                      isOutput=False)
    d_wp = nc.declare_dram_parameter("wp", [P, nt * C], BF16, isOutput=False)
    d_outT = nc.declare_dram_parameter("outT", [C, b_shard], BF16, isOutput=True)

    with tile.TileContext(nc) as tc:
        with (
            tc.tile_pool(name="consts", bufs=1) as consts,
            tc.tile_pool(name="bigbuf", bufs=1) as bigbuf,
            tc.tile_pool(name="scratch", bufs=2) as scratch,
            tc.tile_pool(name="psum_L", bufs=1, space="PSUM") as psum_L,
            tc.tile_pool(name="psum_acc", bufs=1, space="PSUM") as psum_acc,
        ):
            xin_sb = consts.tile([F, xin_cols], mybir.dt.uint8)
            nc.sync.dma_start(out=xin_sb, in_=d_xin[:, :])
            x_sb = xin_sb[:, 0:4 * b_shard].bitcast(F32)
            inc_sb = xin_sb[:, 4 * b_shard:].bitcast(F16)
            wp_sb = consts.tile([P, nt * C], BF16)
            ncols = nt * C
            step = -(-ncols // WP_DMA_SPLIT)
            for c0 in range(0, ncols, step):
                c1 = min(c0 + step, ncols)
                nc.sync.dma_start(out=wp_sb[:, c0:c1], in_=d_wp[:, c0:c1])

            for _rep in range(repeat):
                _body(nc, tc, consts, bigbuf, scratch, psum_L, psum_acc,
                      d_outT, x_sb, inc_sb, wp_sb, F, C, b_shard, nt)
    nc.finalize()
    return nc


def _body(nc, tc, consts, bigbuf, scratch, psum_L, psum_acc, d_outT,
          x_sb, inc_sb, wp_sb, F, C, b_shard, nt):
    # rhs16 = ln(max(|x|,eps)) fp16; rhs_s = (x<0) fp16; both channels
    # contract against the same incidence tile.
    rhs16 = scratch.tile([F, b_shard], F16, tag="rhs")
    rhs_s = scratch.tile([F, b_shard], F16, tag="rhs_s")
    ax = scratch.tile([F, b_shard], F32, tag="ax")
    nc.scalar.activation(ax, x_sb, mybir.ActivationFunctionType.Abs)
    axc = scratch.tile([F, b_shard], F32, tag="axc")
    nc.vector.tensor_scalar(
        out=axc, in0=ax, scalar1=LOG_CLAMP, scalar2=None,
        op0=mybir.AluOpType.max)
    nc.scalar.activation(rhs16, axc, mybir.ActivationFunctionType.Ln)
    nc.vector.tensor_scalar(
        out=rhs_s, in0=x_sb, scalar1=0.0, scalar2=None,
        op0=mybir.AluOpType.is_lt)

    prods = bigbuf.tile([P, nt * b_shard], BF16, tag="prods")
    sig = bigbuf.tile([P, nt * b_shard], BF16, tag="sig")

    # Both channels of a 3-tile round share ONE 6-bank PSUM tile and ONE
    # accumulation group (6 matmuls, one stop): start=True clears each
    # bank, only the last matmul carries stop, so the whole round costs a
    # single consumer semaphore.  Columns [0:3b) = L tiles, [3b:6b) = par.
    t = 0
    while t < nt:
        g = min(GROUP, nt - t)
        Lp = psum_L.tile([P, 2 * GROUP * b_shard], F32, tag="L")
        nmm = 0
        for j in range(g):
            nc.tensor.matmul(
                Lp[:, j * b_shard:(j + 1) * b_shard],
                inc_sb[:, (t + j) * P:(t + j + 1) * P],
                rhs16,
                start=True, stop=False, skip_group_check=True)
        for j in range(g):
            nc.tensor.matmul(
                Lp[:, (GROUP + j) * b_shard:(GROUP + j + 1) * b_shard],
                inc_sb[:, (t + j) * P:(t + j + 1) * P],
                rhs_s,
                start=True, stop=(j == g - 1), skip_group_check=True)
        nc.scalar.activation(
            prods[:, t * b_shard:(t + g) * b_shard], Lp[:, :g * b_shard],
            mybir.ActivationFunctionType.Exp)
        nc.vector.tensor_scalar(
            out=sig[:, t * b_shard:(t + g) * b_shard],
            in0=Lp[:, GROUP * b_shard:(GROUP + g) * b_shard],
            scalar1=-2.0, scalar2=1.0,
            op0=mybir.AluOpType.mult, op1=mybir.AluOpType.add)
        t += g

    # sign fold in quarters so the contraction can start early.
    # sig holds 1-2*par in {1,-1,-3,-5}; adding 4*(par>=2) maps it to
    # (-1)^par in {1,-1} exactly.
    nq = 4
    bounds = [(nt * q // nq) * b_shard for q in range(nq + 1)]
    qmax = max(b - a for a, b in zip(bounds, bounds[1:]))
    htmp = bigbuf.tile([P, qmax], BF16, tag="htmp")
    for lo, hi in zip(bounds, bounds[1:]):
        n = hi - lo
        nc.vector.tensor_scalar(
            out=htmp[:, :n], in0=sig[:, lo:hi], scalar1=-2.5, scalar2=4.0,
            op0=mybir.AluOpType.is_le, op1=mybir.AluOpType.mult)
        nc.vector.tensor_add(
            out=sig[:, lo:hi], in0=sig[:, lo:hi], in1=htmp[:, :n])
        nc.vector.tensor_mul(
            out=prods[:, lo:hi], in0=prods[:, lo:hi], in1=sig[:, lo:hi])

    acc = psum_acc.tile([C, b_shard], F32)
    for t2 in range(nt):
        nc.tensor.matmul(
            acc,
            wp_sb[:, t2 * C:(t2 + 1) * C],
            prods[:, t2 * b_shard:(t2 + 1) * b_shard],
            start=(t2 == 0), stop=(t2 == nt - 1))

    out_sb = bigbuf.tile([C, b_shard], BF16, tag="out")
    nc.vector.tensor_copy(out=out_sb, in_=acc)
    nc.sync.dma_start(out=d_outT[:, :], in_=out_sb)


_nc_cache = {}


def _get_nc(F, C, b_shard, nt, repeat=1):
    key = (F, C, b_shard, nt, repeat)
    if key not in _nc_cache:
        _nc_cache[key] = _build_nc(F, C, b_shard, nt, repeat)
    return _nc_cache[key]


def _make_in_maps(x, inc16, wp_dev, b_shard):
    F = x.shape[1]
    incb = np.ascontiguousarray(inc16).view(np.uint8)
    in_maps = []
    for i in range(N_CORES):
        sh = np.ascontiguousarray(
            x[i * b_shard:(i + 1) * b_shard].T.astype(np.float32))
        blob = np.concatenate([sh.view(np.uint8), incb], axis=1)
        in_maps.append({"xin": np.ascontiguousarray(blob), "wp": wp_dev})
    return in_maps


def kernel(x, bias, W1, W2, W3, idx1, idx2, idx3, _trace=False):
    x = np.asarray(x, np.float32)
    B, F = x.shape
    C = np.asarray(W1).shape[1]
    assert B % N_CORES == 0
    b_shard = B // N_CORES

    inc16, wp_dev, nt = _build_tables(W1, W2, W3, idx1, idx2, idx3, F)
    nc = _get_nc(F, C, b_shard, nt)
    in_maps = _make_in_maps(x, inc16, wp_dev, b_shard)
    res = run_bass_kernel_spmd(nc, in_maps, list(range(N_CORES)), trace=_trace)
    out = np.empty((B, C), np.float32)
    for i in range(N_CORES):
        o = np.asarray(res.results[i]["outT"]).astype(np.float32)
        out[i * b_shard:(i + 1) * b_shard] = o.T
    out += np.asarray(bias, np.float32).reshape(1, -1)
    if _trace:
        kernel.last_results = res
    return out


# revision 20
# speedup vs baseline: 1.0934x; 1.0934x over previous
"""Trainium2 Bass kernel for nn_Linear_27608049779368.

Reference computation:
    out[b,c] = bias[c] + sum_o prod(x[:, idx_o], axis=2) @ W_o
    x [4096, 32], orders 1..3 with 32/496/4960 combos, C=128 classes.

Data-parallel over batch: 8 cores x 512 rows each.

On this stack the cost is dominated by per-instruction dispatch
(~16us per PE LDWEIGHTS/MATMUL, ~120us per PSUM accumulation-group
stop, ~0.3-2ms per DMA) and NEFF DMA bytes (~GB/s-scale), not by
engine cycles, so the kernel minimizes instructions / groups / bytes:

  1. TRUE products via log-magnitude + sign-parity (no c-shift, so no
     cancellation blow-up and bf16 weights suffice):
       magnitude channel: 43 fp16 matmuls  L_t = inc_t.T @ ln(max(|x|,eps))
       parity channel:    43 fp16 matmuls  par_t = inc_t.T @ (x<0)
     Both share one fp16 incidence table (counts, exact in fp16).
  2. Each PSUM tile holds GROUP=3 matmul outputs written as ONE
     accumulation group (start=True clears each bank; stop only on the
     last matmul -> one consumer semaphore per group instead of three).
  3. Evacuation: ACT Exp(L) -> prods (bf16); DVE 1-2*par -> sig.  A
     quartered fold sig += 4*(par>=2); prods *= sig applies exact
     (-1)^par signs while letting the contraction start early.
  4. Contraction: 43 bf16 matmuls (lhsT = [W1;W2;W3] tiles, rhs = prods
     tiles) chained into a single PSUM bank accumulation group.
  5. DMA: one uint8 blob (x f32 + incidence fp16, 0.41MB) that unblocks
     the product matmuls quickly, the bf16 weights (1.41MB) overlapped,
     one bf16 output (0.13MB).  bias is added on the host.

Accuracy: fp16 ln|x| operands + bf16 products/weights -> rel err
~2.9e-3 against the fp32 reference (budget 2e-2).  Measured per-body
repeat-delta 6.9-15ms across device drift vs 35.9ms for the previous
kernel on the same meter (harness baseline 23.9ms).
"""

import os
import sys

import numpy as np

for _p in ("/opt/trn_rl_repo", "/root/.axon_site/_ro/trn_rl_repo"):
    if os.path.isdir(_p) and _p not in sys.path:
        sys.path.insert(0, _p)
        break

import concourse.bass as bass
import concourse.bacc as bacc
import concourse.tile as tile
from concourse import mybir
from concourse.bass_utils import run_bass_kernel_spmd

N_CORES = 8
P = 128
F32 = mybir.dt.float32
F16 = mybir.dt.float16
BF16 = mybir.dt.bfloat16

GROUP = 3          # product tiles per PSUM evacuation group (3 banks)
WP_DMA_SPLIT = 2   # number of chunks for the weight DMA
LOG_CLAMP = 1e-8


# ----------------------------------------------------------------------------
# Host-side prep
# ----------------------------------------------------------------------------

def _build_tables(W1, W2, W3, idx1, idx2, idx3, F):
    """Incidence counts (fp16) and weights (bf16, SBUF tile layout)."""
    idxs = [np.asarray(idx1), np.asarray(idx2), np.asarray(idx3)]
    Ws = [np.asarray(W1), np.asarray(W2), np.asarray(W3)]
    C = Ws[0].shape[1]
    NK = sum(i.shape[0] for i in idxs)
    nt = -(-NK // P)
    NKp = nt * P

    inc = np.zeros((F, NKp), np.float32)
    col = 0
    for idx in idxs:
        n, o = idx.shape
        cols = np.arange(col, col + n)
        for j in range(o):
            np.add.at(inc, (idx[:, j], cols), 1.0)
        col += n

    inc16 = np.ascontiguousarray(inc, dtype=np.float16)

    Wp = np.zeros((NKp, C), np.float32)
    Wp[:NK] = np.vstack([w.astype(np.float32) for w in Ws])
    # SBUF layout: wp_dev[p, t*C + c] = Wp[t*P + p, c]
    import ml_dtypes
    wp_dev = np.ascontiguousarray(
        Wp.reshape(nt, P, C).transpose(1, 0, 2).reshape(P, nt * C)
    ).astype(ml_dtypes.bfloat16)
    return inc16, wp_dev, nt


# ----------------------------------------------------------------------------
# Device kernel
# ----------------------------------------------------------------------------

def _build_nc(F, C, b_shard, nt, repeat=1):
    # xin blob: per partition (0..F) 4*b_shard bytes of x (f32) then
    # 2*nt*P bytes of incidence (fp16)
    xin_cols = 4 * b_shard + 2 * nt * P
    nc = bacc.Bacc(None, target_bir_lowering=False)
    d_xin = nc.declare_dram_parameter("xin", [F, xin_cols], mybir.dt.uint8,
                                      isOutput=False)
    d_wp = nc.declare_dram_parameter("wp", [P, nt * C], BF16, isOutput=False)
    d_outT = nc.declare_dram_parameter("outT", [C, b_shard], BF16, isOutput=True)

    with tile.TileContext(nc) as tc:
        with (
            tc.tile_pool(name="consts", bufs=1) as consts,
            tc.tile_pool(name="bigbuf", bufs=1) as bigbuf,
            tc.tile_pool(name="scratch", bufs=2) as scratch,
            tc.tile_pool(name="psum_L", bufs=1, space="PSUM") as psum_L,
            tc.tile_pool(name="psum_acc", bufs=1, space="PSUM") as psum_acc,
        ):
            xin_sb = consts.tile([F, xin_cols], mybir.dt.uint8)
            nc.sync.dma_start(out=xin_sb, in_=d_xin[:, :])
            x_sb = xin_sb[:, 0:4 * b_shard].bitcast(F32)
            inc_sb = xin_sb[:, 4 * b_shard:].bitcast(F16)
            wp_sb = consts.tile([P, nt * C], BF16)
            ncols = nt * C
            step = -(-ncols // WP_DMA_SPLIT)
            for c0 in range(0, ncols, step):
                c1 = min(c0 + step, ncols)
                nc.sync.dma_start(out=wp_sb[:, c0:c1], in_=d_wp[:, c0:c1])

            for _rep in range(repeat):
                _body(nc, tc, consts, bigbuf, scratch, psum_L, psum_acc,
                      d_outT, x_sb, inc_sb, wp_sb, F, C, b_shard, nt)
    nc.finalize()
    return nc


def _body(nc, tc, consts, bigbuf, scratch, psum_L, psum_acc, d_outT,
          x_sb, inc_sb, wp_sb, F, C, b_shard, nt):
    # rhs16 = ln(max(|x|,eps)) fp16; rhs_s = (x<0) fp16; both channels
    # contract against the same incidence tile.
    rhs16 = scratch.tile([F, b_shard], F16, tag="rhs")
    rhs_s = scratch.tile([F, b_shard], F16, tag="rhs_s")
    ax = scratch.tile([F, b_shard], F32, tag="ax")
    nc.scalar.activation(ax, x_sb, mybir.ActivationFunctionType.Abs)
    axc = scratch.tile([F, b_shard], F32, tag="axc")
    nc.vector.tensor_scalar(
        out=axc, in0=ax, scalar1=LOG_CLAMP, scalar2=None,
        op0=mybir.AluOpType.max)
    nc.scalar.activation(rhs16, axc, mybir.ActivationFunctionType.Ln)
    nc.vector.tensor_scalar(
        out=rhs_s, in0=x_sb, scalar1=0.0, scalar2=None,
        op0=mybir.AluOpType.is_lt)

    prods = bigbuf.tile([P, nt * b_shard], BF16, tag="prods")
    sig = bigbuf.tile([P, nt * b_shard], BF16, tag="sig")

    # Both channels of a 3-tile round share ONE 6-bank PSUM tile and ONE
    # accumulation group (6 matmuls, one stop): start=True clears each
    # bank, only the last matmul carries stop, so the whole round costs a
    # single consumer semaphore.  Columns [0:3b) = L tiles, [3b:6b) = par.
    t = 0
    while t < nt:
        g = min(GROUP, nt - t)
        Lp = psum_L.tile([P, 2 * GROUP * b_shard], F32, tag="L")
        for j in range(g):
            nc.tensor.matmul(
                Lp[:, j * b_shard:(j + 1) * b_shard],
                inc_sb[:, (t + j) * P:(t + j + 1) * P],
                rhs16,
                start=True, stop=False, skip_group_check=True)
        for j in range(g):
            nc.tensor.matmul(
                Lp[:, (GROUP + j) * b_shard:(GROUP + j + 1) * b_shard],
                inc_sb[:, (t + j) * P:(t + j + 1) * P],
                rhs_s,
                start=True, stop=(j == g - 1), skip_group_check=True)
        nc.scalar.activation(
            prods[:, t * b_shard:(t + g) * b_shard], Lp[:, :g * b_shard],
            mybir.ActivationFunctionType.Exp)
        nc.vector.tensor_scalar(
            out=sig[:, t * b_shard:(t + g) * b_shard],
            in0=Lp[:, GROUP * b_shard:(GROUP + g) * b_shard],
            scalar1=-2.0, scalar2=1.0,
            op0=mybir.AluOpType.mult, op1=mybir.AluOpType.add)
        t += g

    # sign fold in quarters so the contraction can start early.
    # sig holds 1-2*par in {1,-1,-3,-5}; adding 4*(par>=2) maps it to
    # (-1)^par in {1,-1} exactly.
    nq = 4
    bounds = [(nt * q // nq) * b_shard for q in range(nq + 1)]
    qmax = max(b - a for a, b in zip(bounds, bounds[1:]))
    htmp = bigbuf.tile([P, qmax], BF16, tag="htmp")
    for lo, hi in zip(bounds, bounds[1:]):
        n = hi - lo
        nc.vector.tensor_scalar(
            out=htmp[:, :n], in0=sig[:, lo:hi], scalar1=-2.5, scalar2=4.0,
            op0=mybir.AluOpType.is_le, op1=mybir.AluOpType.mult)
        nc.vector.tensor_add(
            out=sig[:, lo:hi], in0=sig[:, lo:hi], in1=htmp[:, :n])
        nc.vector.tensor_mul(
            out=prods[:, lo:hi], in0=prods[:, lo:hi], in1=sig[:, lo:hi])

    acc = psum_acc.tile([C, b_shard], F32)
    for t2 in range(nt):
        nc.tensor.matmul(
            acc,
            wp_sb[:, t2 * C:(t2 + 1) * C],
            prods[:, t2 * b_shard:(t2 + 1) * b_shard],
            start=(t2 == 0), stop=(t2 == nt - 1))

    out_sb = bigbuf.tile([C, b_shard], BF16, tag="out")
    nc.vector.tensor_copy(out=out_sb, in_=acc)
    nc.sync.dma_start(out=d_outT[:, :], in_=out_sb)


_nc_cache = {}


def _get_nc(F, C, b_shard, nt, repeat=1):
    key = (F, C, b_shard, nt, repeat)
    if key not in _nc_cache:
        _nc_cache[key] = _build_nc(F, C, b_shard, nt, repeat)
    return _nc_cache[key]


def _make_in_maps(x, inc16, wp_dev, b_shard):
    F = x.shape[1]
    incb = np.ascontiguousarray(inc16).view(np.uint8)
    in_maps = []
    for i in range(N_CORES):
        sh = np.ascontiguousarray(
            x[i * b_shard:(i + 1) * b_shard].T.astype(np.float32))
        blob = np.concatenate([sh.view(np.uint8), incb], axis=1)
        in_maps.append({"xin": np.ascontiguousarray(blob), "wp": wp_dev})
    return in_maps


def kernel(x, bias, W1, W2, W3, idx1, idx2, idx3, _trace=False):
    x = np.asarray(x, np.float32)
    B, F = x.shape
    C = np.asarray(W1).shape[1]
    assert B % N_CORES == 0
    b_shard = B // N_CORES

    inc16, wp_dev, nt = _build_tables(W1, W2, W3, idx1, idx2, idx3, F)
    nc = _get_nc(F, C, b_shard, nt)
    in_maps = _make_in_maps(x, inc16, wp_dev, b_shard)
    res = run_bass_kernel_spmd(nc, in_maps, list(range(N_CORES)), trace=_trace)
    out = np.empty((B, C), np.float32)
    for i in range(N_CORES):
        o = np.asarray(res.results[i]["outT"]).astype(np.float32)
        out[i * b_shard:(i + 1) * b_shard] = o.T
    out += np.asarray(bias, np.float32).reshape(1, -1)
    if _trace:
        kernel.last_results = res
    return out


# revision 23
# speedup vs baseline: 1.6421x; 1.5019x over previous
"""Trainium2 Bass kernel for nn_Linear_27608049779368.

Reference computation:
    out[b,c] = bias[c] + sum_o prod(x[:, idx_o], axis=2) @ W_o
    x [4096, 32], orders 1..3 with 32/496/4960 combos, C=128 classes.

Data-parallel over batch: 8 cores x 512 rows each.

On this stack the cost is dominated by per-instruction dispatch
(~16us per PE LDWEIGHTS/MATMUL, ~120us per PSUM accumulation-group
stop, ~0.3-2ms per DMA) and NEFF DMA bytes (~GB/s-scale), not by
engine cycles, so the kernel minimizes instructions / groups / bytes:

  1. TRUE products via log-magnitude + sign-parity (no c-shift, so no
     cancellation blow-up and bf16 weights suffice):
       magnitude channel: 43 fp16 matmuls  L_t = inc_t.T @ ln(max(|x|,eps))
       parity channel:    43 fp16 matmuls  par_t = inc_t.T @ (x<0)
     Both share one fp16 incidence table (counts, exact in fp16).
  2. Each round packs BOTH channels of 4 tiles (8 matmuls) into ONE
     8-bank PSUM tile as a single accumulation group (start=True clears
     each bank; stop only on the last matmul) -> 11 product group-stops
     instead of 86, the dominant sync cost on this stack.  Parity
     matmuls go first (they only need the cheap is_lt rhs, not the Ln).
     The contraction accumulator time-shares the same pool slot, since
     products and the contraction never overlap.
  3. Evacuation: ACT Exp(L) -> prods (bf16); DVE 1-2*par -> sig.  A
     quartered fold sig += 4*(par>=2); prods *= sig applies exact
     (-1)^par signs while letting the contraction start early.
  4. Contraction: 43 bf16 matmuls (lhsT = [W1;W2;W3] tiles, rhs = prods
     tiles) chained into a single PSUM bank accumulation group.
  5. DMA: one uint8 blob (x f32 + incidence fp16, 0.41MB) that unblocks
     the product matmuls quickly, the bf16 weights (1.41MB) overlapped,
     one bf16 output (0.13MB).  bias is added on the host.

Accuracy: fp16 ln|x| operands + bf16 products/weights -> rel err
~2.9e-3 against the fp32 reference (budget 2e-2).  Measured per-body
repeat-delta ~9.7ms vs 11.7-12.9ms for the 30-stop variant in the same
process, and ~7-15ms across device drift windows; the previous kernel
measured 35.9ms on the same meter (harness baseline 23.9ms).
"""

import os
import sys

import numpy as np

for _p in ("/opt/trn_rl_repo", "/root/.axon_site/_ro/trn_rl_repo"):
    if os.path.isdir(_p) and _p not in sys.path:
        sys.path.insert(0, _p)
        break

import concourse.bass as bass
import concourse.bacc as bacc
import concourse.tile as tile
from concourse import mybir
from concourse.bass_utils import run_bass_kernel_spmd

N_CORES = 8
P = 128
F32 = mybir.dt.float32
F16 = mybir.dt.float16
BF16 = mybir.dt.bfloat16

GROUP = 4          # product tiles per round (2*GROUP PSUM banks per round)
WP_DMA_SPLIT = 2   # number of chunks for the weight DMA
LOG_CLAMP = 1e-8


# ----------------------------------------------------------------------------
# Host-side prep
# ----------------------------------------------------------------------------

def _build_tables(W1, W2, W3, idx1, idx2, idx3, F):
    """Incidence counts (fp16) and weights (bf16, SBUF tile layout)."""
    idxs = [np.asarray(idx1), np.asarray(idx2), np.asarray(idx3)]
    Ws = [np.asarray(W1), np.asarray(W2), np.asarray(W3)]
    C = Ws[0].shape[1]
    NK = sum(i.shape[0] for i in idxs)
    nt = -(-NK // P)
    NKp = nt * P

    inc = np.zeros((F, NKp), np.float32)
    col = 0
    for idx in idxs:
        n, o = idx.shape
        cols = np.arange(col, col + n)
        for j in range(o):
            np.add.at(inc, (idx[:, j], cols), 1.0)
        col += n

    inc16 = np.ascontiguousarray(inc, dtype=np.float16)

    Wp = np.zeros((NKp, C), np.float32)
    Wp[:NK] = np.vstack([w.astype(np.float32) for w in Ws])
    # SBUF layout: wp_dev[p, t*C + c] = Wp[t*P + p, c]
    import ml_dtypes
    wp_dev = np.ascontiguousarray(
        Wp.reshape(nt, P, C).transpose(1, 0, 2).reshape(P, nt * C)
    ).astype(ml_dtypes.bfloat16)
    return inc16, wp_dev, nt


# ----------------------------------------------------------------------------
# Device kernel
# ----------------------------------------------------------------------------

def _build_nc(F, C, b_shard, nt, repeat=1):
    # xin blob: per partition (0..F) 4*b_shard bytes of x (f32) then
    # 2*nt*P bytes of incidence (fp16)
    xin_cols = 4 * b_shard + 2 * nt * P
    nc = bacc.Bacc(None, target_bir_lowering=False)
    d_xin = nc.declare_dram_parameter("xin", [F, xin_cols], mybir.dt.uint8,
                                      isOutput=False)
    d_wp = nc.declare_dram_parameter("wp", [P, nt * C], BF16, isOutput=False)
    d_outT = nc.declare_dram_parameter("outT", [C, b_shard], BF16, isOutput=True)

    with tile.TileContext(nc) as tc:
        with (
            tc.tile_pool(name="consts", bufs=1) as consts,
            tc.tile_pool(name="bigbuf", bufs=1) as bigbuf,
            tc.tile_pool(name="scratch", bufs=2) as scratch,
            tc.tile_pool(name="psum_L", bufs=1, space="PSUM") as psum_L,
        ):
            xin_sb = consts.tile([F, xin_cols], mybir.dt.uint8)
            nc.sync.dma_start(out=xin_sb, in_=d_xin[:, :])
            x_sb = xin_sb[:, 0:4 * b_shard].bitcast(F32)
            inc_sb = xin_sb[:, 4 * b_shard:].bitcast(F16)
            wp_sb = consts.tile([P, nt * C], BF16)
            ncols = nt * C
            step = -(-ncols // WP_DMA_SPLIT)
            for c0 in range(0, ncols, step):
                c1 = min(c0 + step, ncols)
                nc.sync.dma_start(out=wp_sb[:, c0:c1], in_=d_wp[:, c0:c1])

            for _rep in range(repeat):
                _body(nc, tc, consts, bigbuf, scratch, psum_L,
                      d_outT, x_sb, inc_sb, wp_sb, F, C, b_shard, nt)
    nc.finalize()
    return nc


def _body(nc, tc, consts, bigbuf, scratch, psum_L, d_outT,
          x_sb, inc_sb, wp_sb, F, C, b_shard, nt):
    # rhs16 = ln(max(|x|,eps)) fp16; rhs_s = (x<0) fp16; both channels
    # contract against the same incidence tile.
    rhs16 = scratch.tile([F, b_shard], F16, tag="rhs")
    rhs_s = scratch.tile([F, b_shard], F16, tag="rhs_s")
    ax = scratch.tile([F, b_shard], F32, tag="ax")
    nc.scalar.activation(ax, x_sb, mybir.ActivationFunctionType.Abs)
    axc = scratch.tile([F, b_shard], F32, tag="axc")
    nc.vector.tensor_scalar(
        out=axc, in0=ax, scalar1=LOG_CLAMP, scalar2=None,
        op0=mybir.AluOpType.max)
    nc.scalar.activation(rhs16, axc, mybir.ActivationFunctionType.Ln)
    nc.vector.tensor_scalar(
        out=rhs_s, in0=x_sb, scalar1=0.0, scalar2=None,
        op0=mybir.AluOpType.is_lt)

    prods = bigbuf.tile([P, nt * b_shard], BF16, tag="prods")
    sig = bigbuf.tile([P, nt * b_shard], BF16, tag="sig")

    # Both channels of a 4-tile round share ONE 8-bank PSUM tile and ONE
    # accumulation group (8 matmuls, one stop): start=True clears each
    # bank, only the last matmul carries stop, so the whole round costs a
    # single consumer semaphore.  Columns [0:4b) = par tiles, [4b:8b) = L.
    # Parity matmuls go first: they only need rhs_s (one cheap is_lt),
    # so they issue while the Ln activation is still producing rhs16.
    t = 0
    while t < nt:
        g = min(GROUP, nt - t)
        Lp = psum_L.tile([P, 2 * GROUP * b_shard], F32, tag="L")
        for j in range(g):
            nc.tensor.matmul(
                Lp[:, j * b_shard:(j + 1) * b_shard],
                inc_sb[:, (t + j) * P:(t + j + 1) * P],
                rhs_s,
                start=True, stop=False, skip_group_check=True)
        for j in range(g):
            nc.tensor.matmul(
                Lp[:, (GROUP + j) * b_shard:(GROUP + j + 1) * b_shard],
                inc_sb[:, (t + j) * P:(t + j + 1) * P],
                rhs16,
                start=True, stop=(j == g - 1), skip_group_check=True)
        nc.scalar.activation(
            prods[:, t * b_shard:(t + g) * b_shard],
            Lp[:, GROUP * b_shard:(GROUP + g) * b_shard],
            mybir.ActivationFunctionType.Exp)
        nc.vector.tensor_scalar(
            out=sig[:, t * b_shard:(t + g) * b_shard],
            in0=Lp[:, :g * b_shard],
            scalar1=-2.0, scalar2=1.0,
            op0=mybir.AluOpType.mult, op1=mybir.AluOpType.add)
        t += g

    # sign fold in quarters so the contraction can start early.
    # sig holds 1-2*par in {1,-1,-3,-5}; adding 4*(par>=2) maps it to
    # (-1)^par in {1,-1} exactly.
    nq = 4
    bounds = [(nt * q // nq) * b_shard for q in range(nq + 1)]
    qmax = max(b - a for a, b in zip(bounds, bounds[1:]))
    htmp = bigbuf.tile([P, qmax], BF16, tag="htmp")
    for lo, hi in zip(bounds, bounds[1:]):
        n = hi - lo
        nc.vector.tensor_scalar(
            out=htmp[:, :n], in0=sig[:, lo:hi], scalar1=-2.5, scalar2=4.0,
            op0=mybir.AluOpType.is_le, op1=mybir.AluOpType.mult)
        nc.vector.tensor_add(
            out=sig[:, lo:hi], in0=sig[:, lo:hi], in1=htmp[:, :n])
        nc.vector.tensor_mul(
            out=prods[:, lo:hi], in0=prods[:, lo:hi], in1=sig[:, lo:hi])

    # the contraction accumulator time-shares the product pool slot (the
    # last product round has been consumed by then)
    acc_tile = psum_L.tile([P, 2 * GROUP * b_shard], F32, tag="L")
    acc = acc_tile[:, 0:b_shard]
    for t2 in range(nt):
        nc.tensor.matmul(
            acc,
            wp_sb[:, t2 * C:(t2 + 1) * C],
            prods[:, t2 * b_shard:(t2 + 1) * b_shard],
            start=(t2 == 0), stop=(t2 == nt - 1))

    out_sb = bigbuf.tile([C, b_shard], BF16, tag="out")
    nc.vector.tensor_copy(out=out_sb, in_=acc)
    nc.sync.dma_start(out=d_outT[:, :], in_=out_sb)


_nc_cache = {}


def _get_nc(F, C, b_shard, nt, repeat=1):
    key = (F, C, b_shard, nt, repeat)
    if key not in _nc_cache:
        _nc_cache[key] = _build_nc(F, C, b_shard, nt, repeat)
    return _nc_cache[key]


def _make_in_maps(x, inc16, wp_dev, b_shard):
    F = x.shape[1]
    incb = np.ascontiguousarray(inc16).view(np.uint8)
    in_maps = []
    for i in range(N_CORES):
        sh = np.ascontiguousarray(
            x[i * b_shard:(i + 1) * b_shard].T.astype(np.float32))
        blob = np.concatenate([sh.view(np.uint8), incb], axis=1)
        in_maps.append({"xin": np.ascontiguousarray(blob), "wp": wp_dev})
    return in_maps


def kernel(x, bias, W1, W2, W3, idx1, idx2, idx3, _trace=False):
    x = np.asarray(x, np.float32)
    B, F = x.shape
    C = np.asarray(W1).shape[1]
    assert B % N_CORES == 0
    b_shard = B // N_CORES

    inc16, wp_dev, nt = _build_tables(W1, W2, W3, idx1, idx2, idx3, F)
    nc = _get_nc(F, C, b_shard, nt)
    in_maps = _make_in_maps(x, inc16, wp_dev, b_shard)
    res = run_bass_kernel_spmd(nc, in_maps, list(range(N_CORES)), trace=_trace)
    out = np.empty((B, C), np.float32)
    for i in range(N_CORES):
        o = np.asarray(res.results[i]["outT"]).astype(np.float32)
        out[i * b_shard:(i + 1) * b_shard] = o.T
    out += np.asarray(bias, np.float32).reshape(1, -1)
    if _trace:
        kernel.last_results = res
    return out


# revision 25
# speedup vs baseline: 1.8246x; 1.1111x over previous
"""Trainium2 Bass kernel for nn_Linear_27608049779368.

Reference computation:
    out[b,c] = bias[c] + sum_o prod(x[:, idx_o], axis=2) @ W_o
    x [4096, 32], orders 1..3 with 32/496/4960 combos, C=128 classes.

Data-parallel over batch: 8 cores x 512 rows each.

On this stack the cost is dominated by per-instruction dispatch
(~16us per PE LDWEIGHTS/MATMUL, ~120us per PSUM accumulation-group
stop, ~0.3-2ms per DMA) and NEFF DMA bytes (~GB/s-scale), not by
engine cycles, so the kernel minimizes instructions / groups / bytes:

  1. TRUE products via a single-channel offset encode (no c-shift, so
     no cancellation blow-up and bf16 weights suffice): rhs rows are
     ln(max(|x|,eps))-1 and 1024*(x<0)+1 (both fp16-exact streams); one
     K=64 matmul per tile against the row-duplicated incidence gives
     L' = L + 1024*par exactly (order terms cancel) -> 43 product
     matmuls total.
  2. Each round packs 8 tiles into ONE 8-bank PSUM tile as a single
     accumulation group (start=True clears each bank; stop only on the
     last matmul) -> 6 product group-stops, the dominant sync cost on
     this stack.  The contraction accumulator time-shares the same pool
     slot, since products and the contraction never overlap.
  3. Decode per round after one PSUM-releasing copy: three is_ge
     thresholds reconstruct -1024*par, giving L for ACT Exp -> prods
     (bf16) and 1-2*par -> sig.  A quartered fold sig += 4*(par>=2);
     prods *= sig applies exact (-1)^par signs while letting the
     contraction start early.
  4. Contraction: 43 bf16 matmuls (lhsT = [W1;W2;W3] tiles, rhs = prods
     tiles) chained into a single PSUM bank accumulation group.
  5. DMA: one uint8 blob (x f32 + incidence fp16, 0.41MB) that unblocks
     the product matmuls quickly, the bf16 weights (1.41MB) overlapped,
     one bf16 output (0.13MB).  bias is added on the host.

Accuracy: fp16 ln|x| operands + bf16 products/weights -> rel err
~2.9e-3 against the fp32 reference (budget 2e-2).  Measured per-body
repeat-delta ~9.7ms vs 11.7-12.9ms for the 30-stop variant in the same
process, and ~7-15ms across device drift windows; the previous kernel
measured 35.9ms on the same meter (harness baseline 23.9ms).
"""

import os
import sys

import numpy as np

for _p in ("/opt/trn_rl_repo", "/root/.axon_site/_ro/trn_rl_repo"):
    if os.path.isdir(_p) and _p not in sys.path:
        sys.path.insert(0, _p)
        break

import concourse.bass as bass
import concourse.bacc as bacc
import concourse.tile as tile
from concourse import mybir
from concourse.bass_utils import run_bass_kernel_spmd

N_CORES = 8
P = 128
F32 = mybir.dt.float32
F16 = mybir.dt.float16
BF16 = mybir.dt.bfloat16

WP_DMA_SPLIT = 2   # number of chunks for the weight DMA
M_ENC = 1024.0     # parity offset; 1024*s+1 exact in fp16
LOG_CLAMP = 1e-8


# ----------------------------------------------------------------------------
# Host-side prep
# ----------------------------------------------------------------------------

def _build_tables(W1, W2, W3, idx1, idx2, idx3, F):
    """Incidence counts (fp16) and weights (bf16, SBUF tile layout)."""
    idxs = [np.asarray(idx1), np.asarray(idx2), np.asarray(idx3)]
    Ws = [np.asarray(W1), np.asarray(W2), np.asarray(W3)]
    C = Ws[0].shape[1]
    NK = sum(i.shape[0] for i in idxs)
    nt = -(-NK // P)
    NKp = nt * P

    inc = np.zeros((F, NKp), np.float32)
    col = 0
    for idx in idxs:
        n, o = idx.shape
        cols = np.arange(col, col + n)
        for j in range(o):
            np.add.at(inc, (idx[:, j], cols), 1.0)
        col += n

    inc16 = np.ascontiguousarray(np.vstack([inc, inc]), dtype=np.float16)

    Wp = np.zeros((NKp, C), np.float32)
    Wp[:NK] = np.vstack([w.astype(np.float32) for w in Ws])
    # SBUF layout: wp_dev[p, t*C + c] = Wp[t*P + p, c]
    import ml_dtypes
    wp_dev = np.ascontiguousarray(
        Wp.reshape(nt, P, C).transpose(1, 0, 2).reshape(P, nt * C)
    ).astype(ml_dtypes.bfloat16)
    return inc16, wp_dev, nt


# ----------------------------------------------------------------------------
# Device kernel
# ----------------------------------------------------------------------------

def _build_nc(F, C, b_shard, nt, repeat=1):
    # xin blob: partitions 0..2F hold the duplicated fp16 incidence
    # (2*nt*P bytes); partitions 0..F additionally hold x (f32) after it
    xin_cols = 2 * nt * P + 4 * b_shard
    nc = bacc.Bacc(None, target_bir_lowering=False)
    d_xin = nc.declare_dram_parameter("xin", [2 * F, xin_cols],
                                      mybir.dt.uint8, isOutput=False)
    d_wp = nc.declare_dram_parameter("wp", [P, nt * C], BF16, isOutput=False)
    d_outT = nc.declare_dram_parameter("outT", [C, b_shard], BF16, isOutput=True)

    with tile.TileContext(nc) as tc:
        with (
            tc.tile_pool(name="consts", bufs=1) as consts,
            tc.tile_pool(name="bigbuf", bufs=1) as bigbuf,
            tc.tile_pool(name="scratch", bufs=2) as scratch,
            tc.tile_pool(name="dec", bufs=1) as dec,
            tc.tile_pool(name="psum_L", bufs=1, space="PSUM") as psum_L,
        ):
            xin_sb = consts.tile([2 * F, xin_cols], mybir.dt.uint8)
            nc.sync.dma_start(out=xin_sb, in_=d_xin[:, :])
            inc_sb = xin_sb[:, 0:2 * nt * P].bitcast(F16)
            x_sb = xin_sb[0:F, 2 * nt * P:].bitcast(F32)
            wp_sb = consts.tile([P, nt * C], BF16)
            ncols = nt * C
            step = -(-ncols // WP_DMA_SPLIT)
            for c0 in range(0, ncols, step):
                c1 = min(c0 + step, ncols)
                nc.sync.dma_start(out=wp_sb[:, c0:c1], in_=d_wp[:, c0:c1])

            for _rep in range(repeat):
                _body(nc, tc, consts, bigbuf, scratch, dec, psum_L,
                      d_outT, x_sb, inc_sb, wp_sb, F, C, b_shard, nt)
    nc.finalize()
    return nc


def _body(nc, tc, consts, bigbuf, scratch, dec, psum_L, d_outT,
          x_sb, inc_sb, wp_sb, F, C, b_shard, nt):
    # Single-channel offset encode: rhs rows [0,F) = ln(max(|x|,eps)) - 1,
    # rows [F,2F) = 1024*(x<0) + 1.  One K=64 matmul per tile against the
    # duplicated incidence gives L' = L + 1024*par (the order terms
    # cancel).  8 tiles per 8-bank PSUM tile, one accumulation group each
    # (start=True clears each bank, stop only on the last matmul) -> 6
    # product group-stops and 43 product matmuls total.
    RG = 8
    rhs16 = scratch.tile([2 * F, b_shard], F16, tag="rhs")
    ax = scratch.tile([F, b_shard], F32, tag="ax")
    nc.scalar.activation(ax, x_sb, mybir.ActivationFunctionType.Abs)
    axc = scratch.tile([F, b_shard], F32, tag="axc")
    nc.vector.tensor_scalar(
        out=axc, in0=ax, scalar1=LOG_CLAMP, scalar2=None,
        op0=mybir.AluOpType.max)
    lx32 = scratch.tile([F, b_shard], F32, tag="lx32")
    nc.scalar.activation(lx32, axc, mybir.ActivationFunctionType.Ln)
    nc.vector.tensor_scalar(
        out=rhs16[0:F], in0=lx32, scalar1=1.0, scalar2=None,
        op0=mybir.AluOpType.subtract)
    nc.vector.tensor_scalar(
        out=rhs16[F:2 * F], in0=x_sb, scalar1=0.0, scalar2=M_ENC,
        op0=mybir.AluOpType.is_lt, op1=mybir.AluOpType.mult)
    nc.vector.tensor_scalar(
        out=rhs16[F:2 * F], in0=rhs16[F:2 * F], scalar1=1.0, scalar2=None,
        op0=mybir.AluOpType.add)

    prods = bigbuf.tile([P, nt * b_shard], BF16, tag="prods")
    sig = bigbuf.tile([P, nt * b_shard], BF16, tag="sig")
    lcp = dec.tile([P, RG * b_shard], F32, tag="lcp")
    e1 = dec.tile([P, RG * b_shard], F32, tag="e1")
    e2 = dec.tile([P, RG * b_shard], F32, tag="e2")

    t = 0
    while t < nt:
        g = min(RG, nt - t)
        n = g * b_shard
        Lp = psum_L.tile([P, RG * b_shard], F32, tag="L")
        for j in range(g):
            nc.tensor.matmul(
                Lp[:, j * b_shard:(j + 1) * b_shard],
                inc_sb[:, (t + j) * P:(t + j + 1) * P],
                rhs16,
                start=True, stop=(j == g - 1), skip_group_check=True)
        # one PSUM-releasing copy, then threshold-decode par from L' in
        # SBUF: e1 = -1024*par, e2 = L, sig-raw = 1 - 2*par
        nc.vector.tensor_copy(out=lcp[:, :n], in_=Lp[:, :n])
        nc.vector.tensor_scalar(
            out=e1[:, :n], in0=lcp[:, :n], scalar1=M_ENC - 500.0,
            scalar2=-M_ENC, op0=mybir.AluOpType.is_ge,
            op1=mybir.AluOpType.mult)
        nc.vector.tensor_scalar(
            out=e2[:, :n], in0=lcp[:, :n], scalar1=2 * M_ENC - 500.0,
            scalar2=-M_ENC, op0=mybir.AluOpType.is_ge,
            op1=mybir.AluOpType.mult)
        nc.vector.tensor_add(out=e1[:, :n], in0=e1[:, :n], in1=e2[:, :n])
        nc.vector.tensor_scalar(
            out=e2[:, :n], in0=lcp[:, :n], scalar1=3 * M_ENC - 500.0,
            scalar2=-M_ENC, op0=mybir.AluOpType.is_ge,
            op1=mybir.AluOpType.mult)
        nc.vector.tensor_add(out=e1[:, :n], in0=e1[:, :n], in1=e2[:, :n])
        nc.vector.tensor_add(out=e2[:, :n], in0=lcp[:, :n], in1=e1[:, :n])
        nc.scalar.activation(
            prods[:, t * b_shard:t * b_shard + n], e2[:, :n],
            mybir.ActivationFunctionType.Exp)
        nc.vector.tensor_scalar(
            out=sig[:, t * b_shard:t * b_shard + n], in0=e1[:, :n],
            scalar1=2.0 / M_ENC, scalar2=1.0,
            op0=mybir.AluOpType.mult, op1=mybir.AluOpType.add)
        t += g

    # sign fold in quarters so the contraction can start early.
    # sig holds 1-2*par in {1,-1,-3,-5}; adding 4*(par>=2) maps it to
    # (-1)^par in {1,-1} exactly.
    nq = 4
    bounds = [(nt * q // nq) * b_shard for q in range(nq + 1)]
    qmax = max(b - a for a, b in zip(bounds, bounds[1:]))
    htmp = bigbuf.tile([P, qmax], BF16, tag="htmp")
    for lo, hi in zip(bounds, bounds[1:]):
        n = hi - lo
        nc.vector.tensor_scalar(
            out=htmp[:, :n], in0=sig[:, lo:hi], scalar1=-2.5, scalar2=4.0,
            op0=mybir.AluOpType.is_le, op1=mybir.AluOpType.mult)
        nc.vector.tensor_add(
            out=sig[:, lo:hi], in0=sig[:, lo:hi], in1=htmp[:, :n])
        nc.vector.tensor_mul(
            out=prods[:, lo:hi], in0=prods[:, lo:hi], in1=sig[:, lo:hi])

    # the contraction accumulator time-shares the product pool slot (the
    # last product round has been consumed by then)
    acc_tile = psum_L.tile([P, RG * b_shard], F32, tag="L")
    acc = acc_tile[:, 0:b_shard]
    for t2 in range(nt):
        nc.tensor.matmul(
            acc,
            wp_sb[:, t2 * C:(t2 + 1) * C],
            prods[:, t2 * b_shard:(t2 + 1) * b_shard],
            start=(t2 == 0), stop=(t2 == nt - 1))

    out_sb = bigbuf.tile([C, b_shard], BF16, tag="out")
    nc.vector.tensor_copy(out=out_sb, in_=acc)
    nc.sync.dma_start(out=d_outT[:, :], in_=out_sb)


_nc_cache = {}


def _get_nc(F, C, b_shard, nt, repeat=1):
    key = (F, C, b_shard, nt, repeat)
    if key not in _nc_cache:
        _nc_cache[key] = _build_nc(F, C, b_shard, nt, repeat)
    return _nc_cache[key]


def _make_in_maps(x, inc16, wp_dev, b_shard):
    F = x.shape[1]
    incb = np.ascontiguousarray(inc16).view(np.uint8)
    in_maps = []
    for i in range(N_CORES):
        sh = np.ascontiguousarray(
            x[i * b_shard:(i + 1) * b_shard].T.astype(np.float32))
        blob = np.zeros((2 * F, incb.shape[1] + 4 * b_shard), np.uint8)
        blob[:, :incb.shape[1]] = incb
        blob[:F, incb.shape[1]:] = sh.view(np.uint8)
        in_maps.append({"xin": blob, "wp": wp_dev})
    return in_maps


def kernel(x, bias, W1, W2, W3, idx1, idx2, idx3, _trace=False):
    x = np.asarray(x, np.float32)
    B, F = x.shape
    C = np.asarray(W1).shape[1]
    assert B % N_CORES == 0
    b_shard = B // N_CORES

    inc16, wp_dev, nt = _build_tables(W1, W2, W3, idx1, idx2, idx3, F)
    nc = _get_nc(F, C, b_shard, nt)
    in_maps = _make_in_maps(x, inc16, wp_dev, b_shard)
    res = run_bass_kernel_spmd(nc, in_maps, list(range(N_CORES)), trace=_trace)
    out = np.empty((B, C), np.float32)
    for i in range(N_CORES):
        o = np.asarray(res.results[i]["outT"]).astype(np.float32)
        out[i * b_shard:(i + 1) * b_shard] = o.T
    out += np.asarray(bias, np.float32).reshape(1, -1)
    if _trace:
        kernel.last_results = res
    return out


# revision 26
# speedup vs baseline: 2.1702x; 1.1894x over previous
"""Trainium2 Bass kernel for nn_Linear_27608049779368.

Reference computation:
    out[b,c] = bias[c] + sum_o prod(x[:, idx_o], axis=2) @ W_o
    x [4096, 32], orders 1..3 with 32/496/4960 combos, C=128 classes.

Data-parallel over batch: 8 cores x 512 rows each.

On this stack the cost is dominated by per-instruction dispatch
(~16us per PE LDWEIGHTS/MATMUL, ~120us per PSUM accumulation-group
stop, ~0.3-2ms per DMA) and NEFF DMA bytes (~GB/s-scale), not by
engine cycles, so the kernel minimizes instructions / groups / bytes:

  1. TRUE products via a single-channel offset encode (no c-shift, so
     no cancellation blow-up and bf16 weights suffice): rhs rows are
     ln(max(|x|,eps))-1 and 1024*(x<0)+1 (both fp16-exact streams); one
     K=64 matmul per tile against the row-duplicated incidence gives
     L' = L + 1024*par exactly (order terms cancel) -> 43 product
     matmuls total.
  2. Each round packs 8 tiles into ONE 8-bank PSUM tile as a single
     accumulation group (start=True clears each bank; stop only on the
     last matmul) -> 6 product group-stops, the dominant sync cost on
     this stack.  The contraction accumulator time-shares the same pool
     slot, since products and the contraction never overlap.
  3. Decode per round after one PSUM-releasing copy: three is_ge
     thresholds reconstruct -1024*par, giving L for ACT Exp -> prods
     (bf16) and 1-2*par -> sig.  A quartered fold sig += 4*(par>=2);
     prods *= sig applies exact (-1)^par signs while letting the
     contraction start early.
  4. Contraction: 43 bf16 matmuls (lhsT = [W1;W2;W3] tiles, rhs = prods
     tiles) chained into a single PSUM bank accumulation group.
  5. DMA: one uint8 blob (x f32 + incidence fp16, 0.41MB) that unblocks
     the product matmuls quickly, the bf16 weights (1.41MB) overlapped,
     one bf16 output (0.13MB).  bias is added on the host.

Accuracy: fp16 ln|x| operands + bf16 products/weights -> rel err
~2.9e-3 against the fp32 reference (budget 2e-2).  Measured per-body
repeat-delta ~9.7ms vs 11.7-12.9ms for the 30-stop variant in the same
process, and ~7-15ms across device drift windows; the previous kernel
measured 35.9ms on the same meter (harness baseline 23.9ms).
"""

import os
import sys

import numpy as np

for _p in ("/opt/trn_rl_repo", "/root/.axon_site/_ro/trn_rl_repo"):
    if os.path.isdir(_p) and _p not in sys.path:
        sys.path.insert(0, _p)
        break

import concourse.bass as bass
import concourse.bacc as bacc
import concourse.tile as tile
from concourse import mybir
from concourse.bass_utils import run_bass_kernel_spmd

N_CORES = 8
P = 128
F32 = mybir.dt.float32
F16 = mybir.dt.float16
BF16 = mybir.dt.bfloat16

WP_DMA_SPLIT = 2   # number of chunks for the weight DMA
M_ENC = 1024.0     # parity offset; 1024*s+1 exact in fp16
LOG_CLAMP = 1e-8


# ----------------------------------------------------------------------------
# Host-side prep
# ----------------------------------------------------------------------------

def _build_tables(W1, W2, W3, idx1, idx2, idx3, F):
    """Incidence counts (fp16) and weights (bf16, SBUF tile layout)."""
    idxs = [np.asarray(idx1), np.asarray(idx2), np.asarray(idx3)]
    Ws = [np.asarray(W1), np.asarray(W2), np.asarray(W3)]
    C = Ws[0].shape[1]
    NK = sum(i.shape[0] for i in idxs)
    nt = -(-NK // P)
    NKp = nt * P

    inc = np.zeros((F, NKp), np.float32)
    col = 0
    for idx in idxs:
        n, o = idx.shape
        cols = np.arange(col, col + n)
        for j in range(o):
            np.add.at(inc, (idx[:, j], cols), 1.0)
        col += n

    inc16 = np.ascontiguousarray(np.vstack([inc, inc]), dtype=np.float16)

    Wp = np.zeros((NKp, C), np.float32)
    Wp[:NK] = np.vstack([w.astype(np.float32) for w in Ws])
    # SBUF layout: wp_dev[p, t*C + c] = Wp[t*P + p, c]
    import ml_dtypes
    wp_dev = np.ascontiguousarray(
        Wp.reshape(nt, P, C).transpose(1, 0, 2).reshape(P, nt * C)
    ).astype(ml_dtypes.bfloat16)
    return inc16, wp_dev, nt


# ----------------------------------------------------------------------------
# Device kernel
# ----------------------------------------------------------------------------

def _build_nc(F, C, b_shard, nt, repeat=1):
    # xin blob: partitions 0..2F hold the duplicated fp16 incidence
    # (2*nt*P bytes); partitions 0..F additionally hold x (f32) after it
    xin_cols = 2 * nt * P + 4 * b_shard
    nc = bacc.Bacc(None, target_bir_lowering=False)
    d_xin = nc.declare_dram_parameter("xin", [2 * F, xin_cols],
                                      mybir.dt.uint8, isOutput=False)
    d_wp = nc.declare_dram_parameter("wp", [P, nt * C], BF16, isOutput=False)
    d_outT = nc.declare_dram_parameter("outT", [C, b_shard], BF16, isOutput=True)

    with tile.TileContext(nc) as tc:
        with (
            tc.tile_pool(name="consts", bufs=1) as consts,
            tc.tile_pool(name="bigbuf", bufs=1) as bigbuf,
            tc.tile_pool(name="scratch", bufs=2) as scratch,
            tc.tile_pool(name="dec", bufs=1) as dec,
            tc.tile_pool(name="dec2", bufs=2) as dec2,
            tc.tile_pool(name="psum_L", bufs=1, space="PSUM") as psum_L,
        ):
            xin_sb = consts.tile([2 * F, xin_cols], mybir.dt.uint8)
            nc.sync.dma_start(out=xin_sb, in_=d_xin[:, :])
            inc_sb = xin_sb[:, 0:2 * nt * P].bitcast(F16)
            x_sb = xin_sb[0:F, 2 * nt * P:].bitcast(F32)
            wp_sb = consts.tile([P, nt * C], BF16)
            ncols = nt * C
            step = -(-ncols // WP_DMA_SPLIT)
            for c0 in range(0, ncols, step):
                c1 = min(c0 + step, ncols)
                nc.sync.dma_start(out=wp_sb[:, c0:c1], in_=d_wp[:, c0:c1])

            for _rep in range(repeat):
                _body(nc, tc, consts, bigbuf, scratch, dec, dec2, psum_L,
                      d_outT, x_sb, inc_sb, wp_sb, F, C, b_shard, nt)
    nc.finalize()
    return nc


def _body(nc, tc, consts, bigbuf, scratch, dec, dec2, psum_L, d_outT,
          x_sb, inc_sb, wp_sb, F, C, b_shard, nt):
    # Single-channel offset encode: rhs rows [0,F) = ln(max(|x|,eps)) - 1,
    # rows [F,2F) = 1024*(x<0) + 1.  One K=64 matmul per tile against the
    # duplicated incidence gives L' = L + 1024*par (the order terms
    # cancel).  8 tiles per 8-bank PSUM tile, one accumulation group each
    # (start=True clears each bank, stop only on the last matmul) -> 6
    # product group-stops and 43 product matmuls total.
    RG = 8
    rhs16 = scratch.tile([2 * F, b_shard], F16, tag="rhs")
    ax = scratch.tile([F, b_shard], F32, tag="ax")
    nc.scalar.activation(ax, x_sb, mybir.ActivationFunctionType.Abs)
    axc = scratch.tile([F, b_shard], F32, tag="axc")
    nc.vector.tensor_scalar(
        out=axc, in0=ax, scalar1=LOG_CLAMP, scalar2=None,
        op0=mybir.AluOpType.max)
    lx32 = scratch.tile([F, b_shard], F32, tag="lx32")
    nc.scalar.activation(lx32, axc, mybir.ActivationFunctionType.Ln)
    nc.vector.tensor_scalar(
        out=rhs16[0:F], in0=lx32, scalar1=1.0, scalar2=None,
        op0=mybir.AluOpType.subtract)
    nc.vector.tensor_scalar(
        out=rhs16[F:2 * F], in0=x_sb, scalar1=0.0, scalar2=M_ENC,
        op0=mybir.AluOpType.is_lt, op1=mybir.AluOpType.mult)
    nc.vector.tensor_scalar(
        out=rhs16[F:2 * F], in0=rhs16[F:2 * F], scalar1=1.0, scalar2=None,
        op0=mybir.AluOpType.add)

    prods = bigbuf.tile([P, nt * b_shard], BF16, tag="prods")
    sig = bigbuf.tile([P, nt * b_shard], BF16, tag="sig")
    e1 = dec.tile([P, RG * b_shard], F32, tag="e1")
    e2 = dec.tile([P, RG * b_shard], F32, tag="e2")

    t = 0
    while t < nt:
        g = min(RG, nt - t)
        n = g * b_shard
        # double-buffered so the PSUM-releasing copy of round r+1 does
        # not wait for round r's threshold reads
        lcp = dec2.tile([P, RG * b_shard], F32, tag="lcp")
        Lp = psum_L.tile([P, RG * b_shard], F32, tag="L")
        for j in range(g):
            nc.tensor.matmul(
                Lp[:, j * b_shard:(j + 1) * b_shard],
                inc_sb[:, (t + j) * P:(t + j + 1) * P],
                rhs16,
                start=True, stop=(j == g - 1), skip_group_check=True)
        # one PSUM-releasing copy, then threshold-decode par from L' in
        # SBUF: e1 = -1024*par, e2 = L, sig-raw = 1 - 2*par
        nc.vector.tensor_copy(out=lcp[:, :n], in_=Lp[:, :n])
        nc.vector.tensor_scalar(
            out=e1[:, :n], in0=lcp[:, :n], scalar1=M_ENC - 500.0,
            scalar2=-M_ENC, op0=mybir.AluOpType.is_ge,
            op1=mybir.AluOpType.mult)
        nc.vector.tensor_scalar(
            out=e2[:, :n], in0=lcp[:, :n], scalar1=2 * M_ENC - 500.0,
            scalar2=-M_ENC, op0=mybir.AluOpType.is_ge,
            op1=mybir.AluOpType.mult)
        nc.vector.tensor_add(out=e1[:, :n], in0=e1[:, :n], in1=e2[:, :n])
        nc.vector.tensor_scalar(
            out=e2[:, :n], in0=lcp[:, :n], scalar1=3 * M_ENC - 500.0,
            scalar2=-M_ENC, op0=mybir.AluOpType.is_ge,
            op1=mybir.AluOpType.mult)
        nc.vector.tensor_add(out=e1[:, :n], in0=e1[:, :n], in1=e2[:, :n])
        nc.vector.tensor_add(out=e2[:, :n], in0=lcp[:, :n], in1=e1[:, :n])
        nc.scalar.activation(
            prods[:, t * b_shard:t * b_shard + n], e2[:, :n],
            mybir.ActivationFunctionType.Exp)
        nc.vector.tensor_scalar(
            out=sig[:, t * b_shard:t * b_shard + n], in0=e1[:, :n],
            scalar1=2.0 / M_ENC, scalar2=1.0,
            op0=mybir.AluOpType.mult, op1=mybir.AluOpType.add)
        t += g

    # sign fold in quarters so the contraction can start early.
    # sig holds 1-2*par in {1,-1,-3,-5}; adding 4*(par>=2) maps it to
    # (-1)^par in {1,-1} exactly.
    nq = 4
    bounds = [(nt * q // nq) * b_shard for q in range(nq + 1)]
    qmax = max(b - a for a, b in zip(bounds, bounds[1:]))
    htmp = bigbuf.tile([P, qmax], BF16, tag="htmp")
    for lo, hi in zip(bounds, bounds[1:]):
        n = hi - lo
        nc.vector.tensor_scalar(
            out=htmp[:, :n], in0=sig[:, lo:hi], scalar1=-2.5, scalar2=4.0,
            op0=mybir.AluOpType.is_le, op1=mybir.AluOpType.mult)
        nc.vector.tensor_add(
            out=sig[:, lo:hi], in0=sig[:, lo:hi], in1=htmp[:, :n])
        nc.vector.tensor_mul(
            out=prods[:, lo:hi], in0=prods[:, lo:hi], in1=sig[:, lo:hi])

    # the contraction accumulator time-shares the product pool slot (the
    # last product round has been consumed by then)
    acc_tile = psum_L.tile([P, RG * b_shard], F32, tag="L")
    acc = acc_tile[:, 0:b_shard]
    for t2 in range(nt):
        nc.tensor.matmul(
            acc,
            wp_sb[:, t2 * C:(t2 + 1) * C],
            prods[:, t2 * b_shard:(t2 + 1) * b_shard],
            start=(t2 == 0), stop=(t2 == nt - 1))

    out_sb = bigbuf.tile([C, b_shard], BF16, tag="out")
    nc.vector.tensor_copy(out=out_sb, in_=acc)
    nc.sync.dma_start(out=d_outT[:, :], in_=out_sb)


_nc_cache = {}


def _get_nc(F, C, b_shard, nt, repeat=1):
    key = (F, C, b_shard, nt, repeat)
    if key not in _nc_cache:
        _nc_cache[key] = _build_nc(F, C, b_shard, nt, repeat)
    return _nc_cache[key]


def _make_in_maps(x, inc16, wp_dev, b_shard):
    F = x.shape[1]
    incb = np.ascontiguousarray(inc16).view(np.uint8)
    in_maps = []
    for i in range(N_CORES):
        sh = np.ascontiguousarray(
            x[i * b_shard:(i + 1) * b_shard].T.astype(np.float32))
        blob = np.zeros((2 * F, incb.shape[1] + 4 * b_shard), np.uint8)
        blob[:, :incb.shape[1]] = incb
        blob[:F, incb.shape[1]:] = sh.view(np.uint8)
        in_maps.append({"xin": blob, "wp": wp_dev})
    return in_maps


def kernel(x, bias, W1, W2, W3, idx1, idx2, idx3, _trace=False):
    x = np.asarray(x, np.float32)
    B, F = x.shape
    C = np.asarray(W1).shape[1]
    assert B % N_CORES == 0
    b_shard = B // N_CORES

    inc16, wp_dev, nt = _build_tables(W1, W2, W3, idx1, idx2, idx3, F)
    nc = _get_nc(F, C, b_shard, nt)
    in_maps = _make_in_maps(x, inc16, wp_dev, b_shard)
    res = run_bass_kernel_spmd(nc, in_maps, list(range(N_CORES)), trace=_trace)
    out = np.empty((B, C), np.float32)
    for i in range(N_CORES):
        o = np.asarray(res.results[i]["outT"]).astype(np.float32)
        out[i * b_shard:(i + 1) * b_shard] = o.T
    out += np.asarray(bias, np.float32).reshape(1, -1)
    if _trace:
        kernel.last_results = res
    return out
